# revision 1
# baseline (speedup 1.0000x reference)
"""Multi-head dense GAT kernel for Trainium2 (8 NeuronCores, batch-parallel).

Problem: x:[8,1024,256] f32, adj:[8,1024,1024] int32{0,1},
         W:[8,64,256] f32 (per-head linear, [out,in]), a:[8,128] f32.
Reference: h = x@W_h^T; e_ij = leakyrelu(a1.h_i + a2.h_j, 0.2); mask adj==0;
           softmax over j; out = elu(attn@h); concat heads -> [8,1024,512].

Math used here (per batch b, head h; s_i = a1.h_i, t_j = a2.h_j, z = s_i+t_j):
  exp(leakyrelu(z)) = exp(z) * max(exp(-0.8 z), 1)
                    = e^{s_i} * max(e^{-0.8 s_i} e^{0.2 t_j}, e^{t_j})
  softmax is invariant to the per-row factor e^{s_i}, so the unnormalized
  masked weight is  S[j,i] = adj[i,j] * max(a'_i * bv_j, v_j)
  with a' = exp(-0.8 s), bv = exp(0.2 t), v = exp(t).
  out[i,:] = elu( (sum_j S[j,i] h[j,:]) / (sum_j S[j,i]) ).
  Also s = x @ (W^T a1), t = x @ (W^T a2) (associativity), so h is only
  needed for the final weighted sum.

Sharding: batch-parallel, core c computes batch element c.
"""

import os
import numpy as np
import ml_dtypes

B, N, D = 8, 1024, 256
H, K = 8, 64
NCORES = 8
P = 128
NJT = N // P          # 8 j-tiles
NIC = N // P          # 8 i-chunks
HPAIRS = H // 2

_CACHED = {}


def _build_nc():
    import concourse.bass as bass
    import concourse.mybir as mybir
    import concourse.tile as tile
    from concourse import bacc
    from concourse.masks import make_identity

    dt = mybir.dt
    Alu = mybir.AluOpType
    Act = mybir.ActivationFunctionType
    AP = bass.AP

    nc = bacc.Bacc(None, target_bir_lowering=False, debug=False)

    # ---- DRAM I/O (per-core shard) ----
    xT = nc.dram_tensor("xT", [D, N], dt.float32, kind="ExternalInput")
    adjT = nc.dram_tensor("adjT", [N, N], dt.bfloat16, kind="ExternalInput")
    w = nc.dram_tensor("w", [H, K, D], dt.float32, kind="ExternalInput")
    wT = nc.dram_tensor("wT", [P, 2, H, K], dt.float32, kind="ExternalInput")
    aT = nc.dram_tensor("aT", [K, H, 2], dt.float32, kind="ExternalInput")
    outT = nc.dram_tensor("outT", [H, N, K], dt.float32, kind="ExternalOutput")

    debug = bool(int(os.environ.get("GAT_DEBUG", "0")))
    if debug:
        dbg_ex = nc.dram_tensor("dbg_ex", [2, 16, N], dt.float32, kind="ExternalOutput")
        dbg_vt = nc.dram_tensor("dbg_vt", [P, NJT, 16], dt.float32, kind="ExternalOutput")
        dbg_abc = nc.dram_tensor("dbg_abc", [P, H, N], dt.bfloat16, kind="ExternalOutput")
        dbg_S = nc.dram_tensor("dbg_S", [P, 2, NJT, N], dt.bfloat16, kind="ExternalOutput")
        dbg_hext = nc.dram_tensor("dbg_hext", [P, NJT, H * 65], dt.bfloat16, kind="ExternalOutput")
        dbg_stage = nc.dram_tensor("dbg_stage", [P, 1024], dt.float32, kind="ExternalOutput")

    with tile.TileContext(nc) as tc:
        with (
            tc.tile_pool(name="const", bufs=1) as constp,
            tc.tile_pool(name="prep", bufs=1) as prep,
            tc.tile_pool(name="big", bufs=1) as big,
            tc.tile_pool(name="spool", bufs=2) as spool,
            tc.tile_pool(name="tp", bufs=3) as tp,
            tc.tile_pool(name="ep", bufs=4) as ep,
            tc.tile_pool(name="po", bufs=4, space="PSUM") as pop,
        ):
            ident = constp.tile([P, P], dt.float32)
            make_identity(nc, ident)
            ones1 = constp.tile([1, P], dt.float32)
            nc.vector.memset(ones1[:], 1.0)
            zb = constp.tile([P, 1], dt.float32)
            nc.vector.memset(zb[:], 0.0)
            m1b = constp.tile([P, 1], dt.float32)
            nc.vector.memset(m1b[:], -1.0)

            # ---- load inputs (ordered by dependency criticality) ----
            w_sb = prep.tile([K, H, D], dt.float32)
            nc.sync.dma_start(w_sb[:], w[:].rearrange("h k d -> k h d"))
            a_sb = prep.tile([K, H, 2], dt.float32)
            nc.sync.dma_start(a_sb[:], aT[:])
            xt_sb = prep.tile([P, 2, N], dt.float32)       # xT d-chunks
            nc.sync.dma_start(xt_sb[:], xT[:].rearrange("(c p) n -> p c n", p=P))
            mT = big.tile([P, NJT, N], dt.bfloat16)        # transposed adj mask
            adjT_r = adjT[:].rearrange("(t p) i -> p t i", p=P)
            nc.sync.dma_start(mT[:, 0, :], adjT_r[:, 0, :])
            wt_sb = prep.tile([P, 2, H, K], dt.float32)
            nc.sync.dma_start(wt_sb[:], wT[:])
            for jt in range(1, NJT):
                nc.sync.dma_start(mT[:, jt, :], adjT_r[:, jt, :])

            # ---- wtilde = W_h^T @ [a1|a2]; psum col c*16 + half*8 + h ----
            ps_w = pop.tile([P, 32], dt.float32, tag="po")
            for h in range(H):
                for c in range(2):
                    for half in range(2):
                        nc.tensor.matmul(
                            ps_w[:, c * 16 + half * 8 + h : c * 16 + half * 8 + h + 1],
                            w_sb[:, h, c * P : (c + 1) * P],
                            a_sb[:, h, half : half + 1],
                            start=True, stop=True,
                        )
            wt2_sb = prep.tile([P, 32], dt.float32)
            nc.vector.tensor_copy(wt2_sb[:], ps_w[:])

            # ---- s_self rows 0-7, s_nb rows 0-7 (separate psums) ----
            ps_ss = pop.tile([8, N], dt.float32, tag="po")
            ps_sn = pop.tile([8, N], dt.float32, tag="po")
            for half in range(2):
                for c in range(2):
                    nc.tensor.matmul(
                        ps_ss[:, half * 512 : (half + 1) * 512],
                        wt2_sb[:, c * 16 : c * 16 + 8],
                        xt_sb[:, c, half * 512 : (half + 1) * 512],
                        start=(c == 0), stop=(c == 1),
                    )
            for half in range(2):
                for c in range(2):
                    nc.tensor.matmul(
                        ps_sn[:, half * 512 : (half + 1) * 512],
                        wt2_sb[:, c * 16 + 8 : c * 16 + 16],
                        xt_sb[:, c, half * 512 : (half + 1) * 512],
                        start=(c == 0), stop=(c == 1),
                    )

            # ---- exp vectors: a' = e^{-0.8 s}, bv = e^{0.2 t}, v = e^{t} ----
            exS = prep.tile([8, N], dt.bfloat16)
            exBV = prep.tile([8, N], dt.float32)
            exVV = prep.tile([8, N], dt.float32)
            nc.scalar.activation(exS[:], ps_ss[:], Act.Exp, bias=zb[:8, :], scale=-0.8)
            nc.scalar.activation(exBV[:], ps_sn[:], Act.Exp, bias=zb[:8, :], scale=0.2)
            nc.scalar.activation(exVV[:], ps_sn[:], Act.Exp, bias=zb[:8, :], scale=1.0)

            # ---- vt: per j-tile transposed scalar columns [128, 16]
            #      col h = bv_h[j], col 8+h = v_h[j]
            vt_sb = prep.tile([P, NJT, 16], dt.float32)
            for jt in range(NJT):
                ps_vt = pop.tile([P, 16], dt.float32, tag="po")
                nc.tensor.transpose(ps_vt[:, 0:8], exBV[:, jt * P : (jt + 1) * P], ident[:8, :8])
                nc.tensor.transpose(ps_vt[:, 8:16], exVV[:, jt * P : (jt + 1) * P], ident[:8, :8])
                nc.vector.tensor_copy(vt_sb[:, jt, :], ps_vt[:])

            # ---- a'-broadcast: exS rows -> DRAM (bf16), then one
            #      broadcast-read DMA per head pair (ACT HWDGE ring) ----
            aScr = nc.dram_tensor("aScr", [H, N], dt.bfloat16, kind="Internal")
            nc.scalar.dma_start(aScr[:], exS[:])
            abc = big.tile([P, H, N], dt.bfloat16)
            for hp in range(HPAIRS):
                nc.scalar.dma_start(
                    abc[:, 2 * hp : 2 * hp + 2, :],
                    AP(aScr[:].tensor, 2 * hp * N, [[0, P], [N, 2], [1, N]]),
                )

            # ---- h-ext per j-tile: [128, H*65] bf16, col h*65+64 stays 1.0 ----
            hext = big.tile([P, NJT, H * 65], dt.bfloat16)
            nc.gpsimd.memset(hext[:], 1.0)
            for jt in range(NJT):
                ps_h = pop.tile([P, 512], dt.float32, tag="po")
                for c in range(2):
                    nc.tensor.matmul(
                        ps_h[:, :],
                        xt_sb[:, c, jt * P : (jt + 1) * P],
                        wt_sb[:, c, :, :],
                        start=(c == 0), stop=(c == 1),
                    )
                nc.scalar.copy(
                    hext[:, jt, :].rearrange("p (h k) -> p h k", h=H)[:, :, 0:K],
                    ps_h[:].rearrange("p (h k) -> p h k", h=H),
                )

            if debug:
                nc.gpsimd.dma_start(dbg_ex[0][0:8], exS[:])
                nc.sync.dma_start(dbg_ex[0][8:16], exBV[:])
                nc.sync.dma_start(dbg_ex[1][0:8], exVV[:])
                nc.sync.dma_start(dbg_vt[:], vt_sb[:])
                nc.sync.dma_start(dbg_abc[:], abc[:])
                nc.sync.dma_start(dbg_hext[:], hext[:])

            # ---- main loop over head pairs (epilogue deferred one pair) ----
            def s_pass(hp, S):
                h0 = 2 * hp
                for jt in range(NJT):
                    t2 = tp.tile([P, 2, N], dt.bfloat16, tag="t2")
                    for hh in range(2):
                        h = h0 + hh
                        nc.vector.tensor_scalar(
                            t2[:, hh, :],
                            abc[:, h, :],
                            vt_sb[:, jt, h : h + 1],
                            vt_sb[:, jt, 8 + h : 8 + h + 1],
                            Alu.mult,
                            Alu.max,
                        )
                    mTb = mT[:, jt, :]
                    nc.vector.tensor_tensor(
                        S[:, :, jt, :],
                        t2[:],
                        AP(mTb.tensor, mTb.offset, [mTb.ap[0], [0, 2], [1, N]]),
                        Alu.mult,
                    )

            def mms(hp, hh, S, ps_o):
                h = 2 * hp + hh
                for ic in range(NIC):
                    off = (ic // 4) * 512 + (ic % 4) * 65
                    for jt in range(NJT):
                        nc.tensor.matmul(
                            ps_o[:, off : off + 65],
                            S[:, hh, jt, ic * P : (ic + 1) * P],
                            hext[:, jt, h * 65 : (h + 1) * 65],
                            start=(jt == 0), stop=(jt == NJT - 1),
                        )

            def epi1(hp, hh, ps_o):
                h = 2 * hp + hh
                rec = ep.tile([P, 8], dt.float32, tag="rec")
                nc.vector.reciprocal(
                    rec[:].rearrange("p (b q) -> p b q", b=2),
                    AP(ps_o.tensor, ps_o.offset + 64, [[1024, P], [512, 2], [65, 4]]),
                )
                stage = ep.tile([P, 512], dt.float32, tag="stage")
                nc.vector.tensor_tensor(
                    stage[:].rearrange("p (b q k) -> p b q k", b=2, q=4),
                    AP(ps_o.tensor, ps_o.offset, [[1024, P], [512, 2], [65, 4], [1, K]]),
                    AP(rec.tensor, rec.offset, [[8, P], [4, 2], [1, 4], [0, K]]),
                    Alu.mult,
                )
                if debug and hp == 0:
                    nc.sync.dma_start(dbg_stage[:, hh * 512 : (hh + 1) * 512], stage[:])
                # elu(y) = relu(y) + exp(min(y,0)) - 1; ACT part here, DVE
                # combine deferred (phase 2) so it never stalls on ACT
                r1 = ep.tile([P, 512], dt.float32, tag="r1")
                nc.scalar.activation(r1[:], stage[:], Act.Relu, bias=zb[:], scale=-1.0)
                nc.scalar.activation(r1[:], r1[:], Act.Exp, bias=zb[:], scale=-1.0)
                nc.scalar.activation(r1[:], r1[:], Act.Identity, bias=m1b[:])
                return (h, stage, r1)

            def epi2(h, stage, r1):
                nc.vector.scalar_tensor_tensor(
                    stage[:], stage[:], 0.0, r1[:], Alu.max, Alu.add,
                )
                nc.scalar.dma_start(
                    outT[h].rearrange("(ic p) k -> p ic k", p=P),
                    stage[:].rearrange("p (ic k) -> p ic k", ic=NIC),
                )

            pend1 = []
            pend2 = []
            for hp in range(HPAIRS):
                S = spool.tile([P, 2, NJT, N], dt.bfloat16, tag="S")
                s_pass(hp, S)
                if debug and hp == 0:
                    nc.sync.dma_start(dbg_S[:], S[:])
                for hh in range(2):
                    ps_o = pop.tile([P, 1024], dt.float32, tag="po")
                    mms(hp, hh, S, ps_o)
                    if len(pend1) >= 2:
                        pend2.append(epi1(*pend1.pop(0)))
                    if len(pend2) >= 1:
                        epi2(*pend2.pop(0))
                    pend1.append((hp, hh, ps_o))
            for args in pend1:
                pend2.append(epi1(*args))
            for args in pend2:
                epi2(*args)

    nc.finalize()
    return nc


def _get_nc():
    if "nc" not in _CACHED:
        _CACHED["nc"] = _build_nc()
    return _CACHED["nc"]


def kernel(x, adj, W, a):
    from concourse.bass_utils import run_bass_kernel_spmd

    x = np.asarray(x)
    adj = np.asarray(adj)
    W = np.asarray(W, dtype=np.float32)
    a = np.asarray(a, dtype=np.float32)

    wT_host = np.ascontiguousarray(W.reshape(H, K, 2, P).transpose(3, 2, 0, 1))
    aT_host = np.ascontiguousarray(a.reshape(H, 2, K).transpose(2, 0, 1))

    in_maps = []
    for c in range(NCORES):
        in_maps.append({
            "xT": np.ascontiguousarray(x[c].T.astype(np.float32)),
            "adjT": np.ascontiguousarray(adj[c].T.astype(ml_dtypes.bfloat16)),
            "w": W,
            "wT": wT_host,
            "aT": aT_host,
        })

    nc = _get_nc()
    res = run_bass_kernel_spmd(
        nc, in_maps, core_ids=list(range(NCORES)),
        trace=bool(int(os.environ.get("GAT_TRACE", "0"))),
    )
    _CACHED["last_results"] = res

    out = np.empty((B, N, H * K), dtype=np.float32)
    for c in range(NCORES):
        oT = res.results[c]["outT"]            # [H, N, K]
        out[c] = oT.transpose(1, 0, 2).reshape(N, H * K)
    return out



# revision 6
# speedup vs baseline: 1.3175x; 1.3175x over previous
"""Multi-head dense GAT kernel for Trainium2 (8 NeuronCores, batch-parallel). v2

Problem: x:[8,1024,256] f32, adj:[8,1024,1024] int32{0,1},
         W:[8,64,256] f32 (per-head linear, [out,in]), a:[8,128] f32.
Reference: h = x@W_h^T; e_ij = leakyrelu(a1.h_i + a2.h_j, 0.2); mask adj==0;
           softmax over j; out = elu(attn@h); concat heads -> [8,1024,512].

Math (per batch b, head h; s_i = a1.h_i, t_j = a2.h_j, z = s_i+t_j):
  exp(leakyrelu(z)) softmax-reduces (dropping the e^{s_i} row factor) to
  S[j,i] = adj[i,j] * max(a'_i * bv_j, v_j),  a' = e^{-0.8 s},
  bv = e^{0.2 t}, v = e^{t}.
  out[i,:] = elu( (sum_j S[j,i] h[j,:]) / (sum_j S[j,i]) ).
  s = x @ (W^T a1), t = x @ (W^T a2); h only needed for the weighted sum.

Engine split (balanced against the TimelineSim cost model):
  - t2 = (a'bcast * bv) max v : tensor_scalar (4x DVE mode); a tunable subset
    of the 64 (head, j-tile) tiles runs on Pool (tensor_scalar) or ACT
    (Relu + Identity 2-pass) instead.
  - S = t2 * mask : DVE tensor_tensor (2x), grouped 4096 wide over
    (2 heads x 2 j-tiles).
  - attn matmul: 65-wide bf16 matmuls (denominator via ones column).
  - epilogue per head, two variants to balance engines:
      A: recip+divide DVE, q=exp ACT, min(q-1,0) + combine Pool
      B: recip DVE, copy ACT, divide Pool, q=exp ACT, min+combine Pool
  - bf16 output, two quad-grouped output DMAs.
"""

import os
import numpy as np
import ml_dtypes

B, N, D = 8, 1024, 256
H, K = 8, 64
NCORES = 8
P = 128
NJT = N // P          # 8 j-tiles
NIC = N // P          # 8 i-chunks
NG = 4                # head groups (2 heads each)

_CACHED = {}


def _spread(n, total):
    """n indices spread evenly over range(total)."""
    if n <= 0:
        return set()
    return {int((i + 0.5) * total / n) for i in range(n)}


def _ts_assignment():
    # Pool has no tensor_scalar on real TRN2 (ISA check) -- ts tiles are
    # DVE or ACT only. ACT tiles sit in groups 1-3 (ACT does hext copies
    # during group 0).
    ka = int(os.environ.get("GAT_KA", "20"))
    acts = {16 + i for i in _spread(ka, 48)}
    return {i: ("act" if i in acts else "dve") for i in range(64)}


def _tt_pool_set():
    # which of the 16 mask tensor_tensors run on Pool (valid: tt mult)
    kp = int(os.environ.get("GAT_KP", "1"))
    return {4 + i for i in _spread(kp, 12)}


def _build_nc():
    import concourse.bass as bass
    import concourse.mybir as mybir
    import concourse.tile as tile
    from concourse import bacc
    from concourse.masks import make_identity

    dt = mybir.dt
    Alu = mybir.AluOpType
    Act = mybir.ActivationFunctionType
    AP = bass.AP

    nc = bacc.Bacc(None, target_bir_lowering=False, debug=False)

    # ---- DRAM I/O (per-core shard) ----
    xT = nc.dram_tensor("xT", [D, N], dt.bfloat16, kind="ExternalInput")
    adjT = nc.dram_tensor("adjT", [N, N], dt.bfloat16, kind="ExternalInput")
    w = nc.dram_tensor("w", [H, K, D], dt.bfloat16, kind="ExternalInput")
    wT = nc.dram_tensor("wT", [P, 2, H, K], dt.bfloat16, kind="ExternalInput")
    aT = nc.dram_tensor("aT", [K, H, 2], dt.bfloat16, kind="ExternalInput")
    outQ = nc.dram_tensor("outQ", [2, N, 4 * K], dt.bfloat16, kind="ExternalOutput")

    ts_engine = _ts_assignment()

    with tile.TileContext(nc) as tc:
        with (
            tc.tile_pool(name="const", bufs=1) as constp,
            tc.tile_pool(name="prep", bufs=1) as prep,
            tc.tile_pool(name="big", bufs=1) as big,
            tc.tile_pool(name="spool", bufs=2) as spool,
            tc.tile_pool(name="tp", bufs=5) as tp,
            tc.tile_pool(name="ep", bufs=4) as ep,
            tc.tile_pool(name="po", bufs=4, space="PSUM") as pop,
        ):
            ident = constp.tile([P, P], dt.float32)
            make_identity(nc, ident)
            zb = constp.tile([P, 1], dt.float32)
            nc.vector.memset(zb[:], 0.0)
            ones_row = constp.tile([1, P], dt.bfloat16)
            nc.vector.memset(ones_row[:], 1.0)

            # ---- load inputs (all on SP queue; order chosen so the
            # abc-broadcast chain and early mask chunks win the single
            # DMA-device FIFO in need order) ----
            w_sb = prep.tile([K, H, D], dt.bfloat16)
            nc.sync.dma_start(w_sb[:], w[:].rearrange("h k d -> k h d"))
            a_sb = prep.tile([K, H, 2], dt.bfloat16)
            nc.sync.dma_start(a_sb[:], aT[:])
            xt_sb = prep.tile([P, 2, N], dt.bfloat16)       # xT d-chunks
            xT_r = xT[:].rearrange("(c p) n -> p c n", p=P)
            nc.sync.dma_start(xt_sb[:, :, 0:512], xT_r[:, :, 0:512])
            nc.sync.dma_start(xt_sb[:, :, 512:1024], xT_r[:, :, 512:1024])
            mT = big.tile([P, NJT, N], dt.bfloat16)
            adjT_r = adjT[:].rearrange("(t p) i -> p t i", p=P)
            wt_sb = prep.tile([P, 2, H, K], dt.bfloat16)
            for jtp in range(2):
                nc.sync.dma_start(
                    mT[:, 2 * jtp : 2 * jtp + 2, :],
                    adjT_r[:, 2 * jtp : 2 * jtp + 2, :])
            nc.sync.dma_start(wt_sb[:], wT[:])
            nc.sync.dma_start(mT[:, 4:6, :], adjT_r[:, 4:6, :])
            nc.sync.dma_start(mT[:, 6:8, :], adjT_r[:, 6:8, :])

            # ---- wtilde = W_h^T @ [a1|a2]; psum col c*16 + half*8 + h ----
            ps_w = pop.tile([P, 32], dt.float32, tag="po")
            for h in range(H):
                for c in range(2):
                    for half in range(2):
                        nc.tensor.matmul(
                            ps_w[:, c * 16 + half * 8 + h : c * 16 + half * 8 + h + 1],
                            w_sb[:, h, c * P : (c + 1) * P],
                            a_sb[:, h, half : half + 1],
                            start=True, stop=True,
                        )
            wt2_sb = prep.tile([P, 32], dt.bfloat16)
            nc.vector.tensor_copy(wt2_sb[:], ps_w[:])

            # ---- s_self: all 8 heads batched at partitions 0-7, plus a
            # duplicate of head 1 at partition 32 so the PE can broadcast
            # heads 0 and 1 directly (base-partition rule: 0/32/64) ----
            ps_ss = pop.tile([8, N], dt.float32, tag="po")
            for half in range(2):
                for c in range(2):
                    nc.tensor.matmul(
                        ps_ss[:, half * 512 : (half + 1) * 512],
                        wt2_sb[:, c * 16 : c * 16 + 8],
                        xt_sb[:, c, half * 512 : (half + 1) * 512],
                        start=(c == 0), stop=(c == 1),
                    )

            exS = prep.tile([8, N], dt.bfloat16)
            nc.scalar.activation(exS[:], ps_ss[:], Act.Exp,
                                 bias=zb[0:8, :], scale=-0.8)

            # ---- t computed directly TRANSPOSED: tT[j, h] per j-tile ----
            ps_tT = pop.tile([P, NJT * 8], dt.float32, tag="po")
            for jt in range(NJT):
                for c in range(2):
                    nc.tensor.matmul(
                        ps_tT[:, jt * 8 : jt * 8 + 8],
                        xt_sb[:, c, jt * P : (jt + 1) * P],
                        wt2_sb[:, c * 16 + 8 : c * 16 + 16],
                        start=(c == 0), stop=(c == 1),
                    )

            # vt: [P, jt, 24]: col h = bv_h[j]=e^{0.2 t}, 8+h = v_h[j]=e^t,
            # 16+h = -v_h[j]
            vt_sb = prep.tile([P, NJT, 24], dt.float32)
            ps_tT_r = ps_tT[:].rearrange("p (t c) -> p t c", t=NJT)
            nc.scalar.activation(vt_sb[:, :, 0:8], ps_tT_r, Act.Exp,
                                 bias=zb[:], scale=0.2)
            nc.scalar.activation(vt_sb[:, :, 8:16], ps_tT_r, Act.Exp,
                                 bias=zb[:], scale=1.0)
            nc.vector.tensor_scalar(
                vt_sb[:, :, 16:24], vt_sb[:, :, 8:16], -1.0, None, Alu.mult)

            # ---- a'-broadcast into abc[P, H, N].
            # Head 0: PE (ones outer exS row 0 -> PSUM -> ACT copy), ready
            # earliest. Heads 1-7: one SBUF->SBUF DMA packs exS rows into
            # partition 0, then GPSIMD partition_broadcast per head (the op
            # requires its source at partition 0; Pool is otherwise idle).
            abc = big.tile([P, H, N], dt.bfloat16)
            ps_b0 = pop.tile([P, 1024], dt.float32, tag="po")
            for c in range(2):
                nc.tensor.matmul(
                    ps_b0[:, c * 512 : (c + 1) * 512],
                    ones_row[:],
                    exS[0:1, c * 512 : (c + 1) * 512],
                    start=True, stop=True,
                )
            nc.scalar.copy(abc[:, 0, :], ps_b0[:])

            exSflat = prep.tile([1, (H - 1) * N], dt.bfloat16)
            nc.sync.dma_start(
                exSflat[:].rearrange("p (h n) -> p h n", h=H - 1),
                exS[1:, :])
            for h in range(1, H):
                nc.gpsimd.partition_broadcast(
                    abc[:, h, :],
                    exSflat[0:1, (h - 1) * N : h * N])

            # ---- h-ext per j-tile: [128, H*65] bf16, col h*65+64 = 1.0 ----
            hext = big.tile([P, NJT, H * 65], dt.bfloat16)
            nc.vector.memset(
                hext[:].rearrange("p t (h c) -> p t h c", h=H)[:, :, :, K : K + 1],
                1.0)
            for jt in range(NJT):
                ps_h = pop.tile([P, 512], dt.float32, tag="po")
                for c in range(2):
                    nc.tensor.matmul(
                        ps_h[:, :],
                        xt_sb[:, c, jt * P : (jt + 1) * P],
                        wt_sb[:, c, :, :],
                        start=(c == 0), stop=(c == 1),
                    )
                nc.scalar.copy(
                    hext[:, jt, :].rearrange("p (h k) -> p h k", h=H)[:, :, 0:K],
                    ps_h[:].rearrange("p (h k) -> p h k", h=H),
                )

            # ---- output staging: [P, ic, h, k] bf16 so quad stores are wide
            stage_all = big.tile([P, NIC, H, K], dt.bfloat16)

            # ---- main loop over head groups (2 heads each) ----
            def s_pass(g, S):
                h0 = 2 * g
                for jtp in range(NJT // 2):
                    t2 = tp.tile([P, 2, 2, N], dt.bfloat16, tag="t2")
                    for hh in range(2):
                        for jj in range(2):
                            jt = 2 * jtp + jj
                            h = h0 + hh
                            idx = g * 16 + jtp * 4 + hh * 2 + jj
                            eng = ts_engine[idx]
                            bv = vt_sb[:, jt, h : h + 1]
                            vv = vt_sb[:, jt, 8 + h : 8 + h + 1]
                            nv = vt_sb[:, jt, 16 + h : 16 + h + 1]
                            if eng == "dve":
                                nc.vector.tensor_scalar(
                                    t2[:, hh, jj, :], abc[:, h, :], bv, vv,
                                    Alu.mult, Alu.max)
                            else:  # act: max(a'bv, v) = relu(bv*a' - v) + v
                                nc.scalar.activation(
                                    t2[:, hh, jj, :], abc[:, h, :], Act.Relu,
                                    bias=nv, scale=bv)
                                nc.scalar.activation(
                                    t2[:, hh, jj, :], t2[:, hh, jj, :],
                                    Act.Identity, bias=vv)
                    # mask multiply, 4096 wide: S[:, hh, 2jtp+jj, :]
                    mTb = mT[:, 2 * jtp, :]
                    nc.vector.tensor_tensor(
                        S[:, :, 2 * jtp : 2 * jtp + 2, :],
                        t2[:],
                        AP(mTb.tensor, mTb.offset,
                           [mTb.ap[0], [0, 2], [N, 2], [1, N]]),
                        Alu.mult,
                    )

            def mms2(g, hh, S, ps_o):
                h = 2 * g + hh
                for ic in range(NIC):
                    off = (ic // 4) * 512 + (ic % 4) * 65
                    for jt in range(NJT):
                        nc.tensor.matmul(
                            ps_o[:, off : off + 65],
                            S[:, hh, jt, ic * P : (ic + 1) * P],
                            hext[:, jt, h * 65 : (h + 1) * 65],
                            start=(jt == 0), stop=(jt == NJT - 1),
                        )

            def epi_front(g, ps_os):
                """DVE part for both heads: reciprocals + divides into y2."""
                y2 = ep.tile([P, 2, 512], dt.bfloat16, tag="y2")
                for hh in range(2):
                    ps_o = ps_os[hh]
                    rec = ep.tile([P, 8], dt.float32, tag="rec",
                                  name=f"rec{g}{hh}")
                    nc.vector.reciprocal(
                        rec[:].rearrange("p (b q) -> p b q", b=2),
                        AP(ps_o.tensor, ps_o.offset + 64,
                           [[1024, P], [512, 2], [65, 4]]),
                    )
                    nc.vector.tensor_tensor(
                        y2[:, hh, :].rearrange("p (b q k) -> p b q k", b=2, q=4),
                        AP(ps_o.tensor, ps_o.offset,
                           [[1024, P], [512, 2], [65, 4], [1, K]]),
                        AP(rec.tensor, rec.offset, [[8, P], [4, 2], [1, 4], [0, K]]),
                        Alu.mult,
                    )
                return y2

            def epi_back(g, y2):
                # elu(y) = max(y, min(e^y - 1, 0))  [median identity]
                q2 = ep.tile([P, 2, 512], dt.bfloat16, tag="q2")
                nc.scalar.activation(q2[:], y2[:], Act.Exp)
                nc.vector.tensor_scalar(q2[:], q2[:], -1.0, 0.0, Alu.add, Alu.min)
                nc.vector.tensor_tensor(
                    stage_all[:, :, 2 * g : 2 * g + 2, :],
                    AP(y2.tensor, y2.offset, [[1024, P], [64, NIC], [512, 2], [1, K]]),
                    AP(q2.tensor, q2.offset, [[1024, P], [64, NIC], [512, 2], [1, K]]),
                    Alu.max,
                )

            def store(g):
                # quad 0 after g=1; pairs for g=2, g=3 (short tail)
                if g == 1:
                    nc.sync.dma_start(
                        outQ[0].rearrange("(ic p) c -> p ic c", p=P),
                        stage_all[:, :, 0:4, :].rearrange("p a h k -> p a (h k)"),
                    )
                elif g >= 2:
                    half = g - 2
                    nc.sync.dma_start(
                        outQ[1, :, half * 128 : half * 128 + 128]
                        .rearrange("(ic p) c -> p ic c", p=P),
                        stage_all[:, :, 2 * g : 2 * g + 2, :]
                        .rearrange("p a h k -> p a (h k)"),
                    )

            def epi_tail(g, hh, b, ps_o):
                """Half-width latency-optimized epilogue for the last group:
                chain DVE recip/divide -> ACT exp -> DVE min -> DVE combine."""
                h = 2 * g + hh
                off = b * 512
                rec = ep.tile([P, 4], dt.float32, tag="rec")
                nc.vector.reciprocal(
                    rec[:],
                    AP(ps_o.tensor, ps_o.offset + off + 64, [[1024, P], [65, 4]]),
                )
                y = ep.tile([P, 256], dt.bfloat16, tag="yh")
                nc.vector.tensor_tensor(
                    y[:].rearrange("p (q k) -> p q k", q=4),
                    AP(ps_o.tensor, ps_o.offset + off, [[1024, P], [65, 4], [1, K]]),
                    AP(rec.tensor, rec.offset, [[4, P], [1, 4], [0, K]]),
                    Alu.mult,
                )
                q = ep.tile([P, 256], dt.bfloat16, tag="qh")
                nc.scalar.activation(q[:], y[:], Act.Exp)
                nc.vector.tensor_scalar(q[:], q[:], -1.0, 0.0, Alu.add, Alu.min)
                nc.vector.tensor_tensor(
                    stage_all[:, 4 * b : 4 * b + 4, h, :],
                    y[:].rearrange("p (a k) -> p a k", a=4),
                    q[:].rearrange("p (a k) -> p a k", a=4),
                    Alu.max,
                )

            # software pipeline: s_pass(g+1) emitted between mms(g) and epi(g)
            # so Pool/ACT offloaded ts-tiles aren't stuck behind epilogue ops
            # in their in-order queues
            S_cur = spool.tile([P, 2, NJT, N], dt.bfloat16, tag="S")
            s_pass(0, S_cur)
            for g in range(NG - 1):
                ps_os = []
                for hh in range(2):
                    ps_o = pop.tile([P, 1024], dt.float32, tag="po",
                                    name=f"ps_o{g}{hh}")
                    mms2(g, hh, S_cur, ps_o)
                    ps_os.append(ps_o)
                S_nxt = spool.tile([P, 2, NJT, N], dt.bfloat16, tag="S")
                s_pass(g + 1, S_nxt)
                y2 = epi_front(g, ps_os)
                epi_back(g, y2)
                store(g)
                S_cur = S_nxt

            # last group: interleave the two heads' matmuls in i-chunk halves
            # so half-width epilogue chains can start at 50% and pipeline
            g = NG - 1
            ps_os = [pop.tile([P, 1024], dt.float32, tag="po", name=f"ps_oL{i}")
                     for i in range(2)]
            for icg in range(2):
                for hh in range(2):
                    h = 2 * g + hh
                    for ic in range(4 * icg, 4 * icg + 4):
                        off = (ic // 4) * 512 + (ic % 4) * 65
                        for jt in range(NJT):
                            nc.tensor.matmul(
                                ps_os[hh][:, off : off + 65],
                                S_cur[:, hh, jt, ic * P : (ic + 1) * P],
                                hext[:, jt, h * 65 : (h + 1) * 65],
                                start=(jt == 0), stop=(jt == NJT - 1),
                            )
            for hh in range(2):
                for b in range(2):
                    epi_tail(g, hh, b, ps_os[hh])
                h = 2 * g + hh
                nc.sync.dma_start(
                    outQ[1, :, 128 + hh * 64 : 192 + hh * 64]
                    .rearrange("(ic p) c -> p ic c", p=P),
                    stage_all[:, :, h, :],
                )

    nc.finalize()
    return nc


def _get_nc():
    if "nc" not in _CACHED:
        _CACHED["nc"] = _build_nc()
    return _CACHED["nc"]


def kernel(x, adj, W, a):
    from concourse.bass_utils import run_bass_kernel_spmd

    x = np.asarray(x)
    adj = np.asarray(adj)
    W = np.asarray(W, dtype=np.float32)
    a = np.asarray(a, dtype=np.float32)

    bf16 = ml_dtypes.bfloat16
    wT_host = np.ascontiguousarray(
        W.reshape(H, K, 2, P).transpose(3, 2, 0, 1).astype(bf16))
    aT_host = np.ascontiguousarray(
        a.reshape(H, 2, K).transpose(2, 0, 1).astype(bf16))
    w_host = W.astype(bf16)

    in_maps = []
    for c in range(NCORES):
        in_maps.append({
            "xT": np.ascontiguousarray(x[c].T.astype(bf16)),
            "adjT": np.ascontiguousarray(adj[c].T.astype(bf16)),
            "w": w_host,
            "wT": wT_host,
            "aT": aT_host,
        })

    nc = _get_nc()
    res = run_bass_kernel_spmd(
        nc, in_maps, core_ids=list(range(NCORES)),
        trace=bool(int(os.environ.get("GAT_TRACE", "0"))),
    )
    _CACHED["last_results"] = res

    out = np.empty((B, N, H * K), dtype=np.float32)
    for c in range(NCORES):
        oQ = res.results[c]["outQ"]            # [2, N, 256] bf16
        out[c, :, 0:256] = oQ[0].astype(np.float32)
        out[c, :, 256:512] = oQ[1].astype(np.float32)
    return out
